# revision 71
# baseline (speedup 1.0000x reference)
"""Cross-attention kernel for 8 TRN2 NeuronCores (v2).

Reference computation (per problem spec):
    q = (x @ Wq)  [B=4, N=4096, D=1024] -> heads [B, 16, N, 64]
    k = (context @ Wk), v = (context @ Wv)   context [B, M=256, 768]
    out = softmax(q k^T / 8 + mask) v   -> [B, N, D] @ Wo

Sharding: the 16384 query rows (B*N) are split evenly across the 8 cores
(2048 rows each, each shard living inside one batch). K/V are computed
redundantly per core from that core's batch context (only ~0.8 GFLOP) so no
collectives are needed; each core produces its own 2048 output rows and the
host concatenates them.

v2 changes vs v1 (282.9us -> ~239us):
  * softmax sums come free from the AV matmul via an all-ones 65th column
    appended to each V-head stationary (kills the 128 ones-matmuls, ~27us
    of PE streaming); the per-(head,q) sums are re-spread across
    partitions with a tiny selector matmul, then reciprocal + normalize.
  * the attention spine is exp(ScalarE)-co-limited, so Qproj of chunk c+1,
    outproj of chunk c-1, and the K/V/Q projections of chunk 0 are
    interleaved into it as just-in-time PE filler groups.
  * x^T: PE-transposed for chunks 0-1 (startup-critical), whole-chunk DMA
    transposes for chunks 2-3 placed last on the sync ring (DMA-transpose
    descriptor generation is ~8us and Tile mutually serializes it against
    every other DMA, so it must stay off the startup path).
  * input DMAs split across sync/scalar rings in consumption order, with
    wk/wq/wv halved so the first half unblocks the PE early; the mask
    rides in as one [1,256] row and is spread across partitions by two
    1-column matmuls (the [128,2] layout's 8-byte descriptors were slow).
  * output stored bf16 (halves store traffic); host upcasts.
"""

import sys

for _p in ("/opt/trn_rl_repo",):
    if _p not in sys.path:
        sys.path.insert(0, _p)

import numpy as np

import concourse.bass as bass
import concourse.mybir as mybir
import concourse.tile as tile
from concourse.masks import make_identity
from concourse import bacc
from concourse.bass_utils import run_bass_kernel_spmd

ts = bass.ts

N_CORES = 8
B, N, D = 4, 4096, 1024
CTX = 768
M = 256          # kv length
H, HD = 16, 64   # heads, head dim
NQ = (B * N) // N_CORES   # 2048 query rows per core
QCH = 512                 # q chunk (free dim of most matmuls)
NQC = NQ // QCH           # 4 q chunks
DT = D // 128             # 8 d-blocks (= head pairs)
KCH = CTX // 128          # 6 contraction chunks for context projections
F32 = mybir.dt.float32
BF16 = mybir.dt.bfloat16

SCALE = HD ** -0.5


def build_nc():
    nc = bacc.Bacc()

    x_ext = nc.declare_dram_parameter("x", [NQ, D], BF16, isOutput=False)
    ctx_ext = nc.declare_dram_parameter("ctx", [M, CTX], BF16, isOutput=False)
    maskb_ext = nc.declare_dram_parameter("maskb", [1, M], F32, isOutput=False)
    wq_ext = nc.declare_dram_parameter("wq", [D, D], BF16, isOutput=False)
    wk_ext = nc.declare_dram_parameter("wk", [CTX, D], BF16, isOutput=False)
    wv_ext = nc.declare_dram_parameter("wv", [CTX, D], BF16, isOutput=False)
    wo_ext = nc.declare_dram_parameter("wo", [D, D], BF16, isOutput=False)
    out_ext = nc.declare_dram_parameter("out", [NQ, D], BF16, isOutput=True)

    with tile.TileContext(nc) as tc:
        # ---- persistent tensors -------------------------------------------
        identb, free_identb = tc.tile([128, 128], BF16, name="identb")
        make_identity(nc, identb)
        mask_sb, free_mask = tc.tile([128, 2], F32, name="mask_sb")
        maskr_sb, free_maskr = tc.tile([1, M], F32, name="maskr_sb")
        ones11, free_ones11 = tc.tile([1, 1], F32, name="ones11")
        warm_sb, free_warm = tc.tile([128, 1], F32, name="warm_sb")
        # selector: row 0 -> output partitions 0-63 (head 2i sums), row 64 ->
        # partitions 64-127 (head 2i+1 sums); all other rows zero.
        sel2, free_sel2 = tc.tile([128, 128], BF16, name="sel2")
        nc.vector.memset(sel2, 0.0)
        nc.vector.memset(sel2[0:1, 0:64], 1.0)
        nc.vector.memset(sel2[64:65, 64:128], 1.0)
        # double-buffered sum-row carriers (only rows 0 and 64 ever written)
        srA, free_srA = tc.tile([128, QCH], BF16, name="srA")
        srB, free_srB = tc.tile([128, QCH], BF16, name="srB")
        nc.vector.memset(srA, 0.0)
        nc.vector.memset(srB, 0.0)

        kT, free_kT = tc.tile([128, DT, M], BF16, name="kT")
        # V stationaries: per (kv-half j, head h) a [128, 65] block whose
        # 65th column is all-ones so the AV matmul also emits softmax sums.
        vvs, free_vvs = tc.tile([128, 2, H, HD + 1], BF16, name="vvs")
        nc.gpsimd.memset(vvs[:, :, :, HD : HD + 1], 1.0)
        xT, free_xT = tc.tile([128, DT, NQ], BF16, name="xT")
        qT, free_qT = tc.tile([128, DT, NQ], BF16, name="qT")
        oT, free_oT = tc.tile([128, DT, NQ], BF16, name="oT")

        with tc.tile_pool(name="weights", bufs=1) as wpool, \
             tc.tile_pool(name="bpool", bufs=1) as bpool, \
             tc.tile_pool(name="xpool", bufs=8) as xpool, \
             tc.tile_pool(name="attnp", bufs=3) as attnp, \
             tc.tile_pool(name="recp", bufs=2) as recp, \
             tc.tile_pool(name="outp", bufs=4) as outp, \
             tc.tile_pool(name="mpsum", bufs=3, space="PSUM") as mpsum:
            # ---- input DMAs (no DMA transposes: Tile mutually serializes
            # them against every other DMA, which stalls the weight loads).
            # sync ring: mask, ctx, wv, x row-blocks; scalar: wq, wk, wo.
            # mask arrives as one contiguous [1, 256] row (a single fast
            # descriptor); a pair of 1-column matmuls spreads it across
            # partitions into the [128, 2] per-partition bias layout.
            nc.sync.dma_start(out=maskr_sb, in_=maskb_ext[:, :])
            nc.vector.memset(ones11, 1.0)
            # p-state warm-up: the PE starts at the 1.2GHz pstate and only
            # ramps with activity (early matmuls measured ~2x slower). Burn
            # the DMA-wait window with data-free transposes so the clock is
            # at full rate when real work arrives.
            for w in range(4):
                tpw = mpsum.tile([128, DT, 128], BF16, name="tp_w", tag="ps")
                for kk in range(DT):
                    nc.tensor.transpose(tpw[:, kk, :], identb, identb)
            mps = mpsum.tile([128, 2], F32, name="mps", tag="ps")
            for j in range(2):
                nc.tensor.matmul(
                    mps[:, j : j + 1], maskr_sb[0:1, ts(j, 128)], ones11,
                    start=True, stop=True,
                )
            nc.vector.tensor_copy(mask_sb, mps)
            # ctx plain load in two halves (DMA-transpose desc-gen is ~7us
            # and blocks the ring; the PE transposes it in ~0.7us instead)
            ctx_sb = bpool.tile([128, 2, CTX], BF16, name="ctx_sb")
            ctx_r = ctx_ext.rearrange("(a p) n -> p a n", p=128)
            nc.sync.dma_start(out=ctx_sb[:, 0:1, :], in_=ctx_r[:, 0:1, :])
            nc.sync.dma_start(out=ctx_sb[:, 1:2, :], in_=ctx_r[:, 1:2, :])
            ctxT = bpool.tile([128, KCH, M], BF16, name="ctxT")
            # weights split into column halves so the first half unblocks
            # the PE early; wk/wq on scalar, wv on sync.
            wk_sb = bpool.tile([128, KCH, D], BF16, name="wk_sb")
            wq_sb = wpool.tile([128, DT, D], BF16, name="wq_sb")
            wv_sb = bpool.tile([128, KCH, D], BF16, name="wv_sb")
            wk_r = wk_ext.rearrange("(a p) n -> p a n", p=128)
            wq_r = wq_ext.rearrange("(a p) n -> p a n", p=128)
            wv_r = wv_ext.rearrange("(a p) n -> p a n", p=128)
            nc.scalar.dma_start(out=wk_sb[:, :, 0:512], in_=wk_r[:, :, 0:512])
            nc.scalar.dma_start(out=wq_sb[:, :, 0:512], in_=wq_r[:, :, 0:512])
            # x row-blocks for chunks 0-1 (plain loads; PE transposes them)
            x_sbs = {}
            for rb in range(4):
                x_sb = xpool.tile([128, D], BF16, name="x_sb", tag="x_sb")
                nc.sync.dma_start(out=x_sb, in_=x_ext[ts(rb, 128), :])
                x_sbs[rb] = x_sb
            nc.sync.dma_start(out=wv_sb[:, :, 0:512], in_=wv_r[:, :, 0:512])
            nc.scalar.dma_start(out=wk_sb[:, :, 512:D], in_=wk_r[:, :, 512:D])
            nc.scalar.dma_start(out=wq_sb[:, :, 512:D], in_=wq_r[:, :, 512:D])
            nc.sync.dma_start(out=wv_sb[:, :, 512:D], in_=wv_r[:, :, 512:D])
            wo_sb = wpool.tile([128, DT, D], BF16, name="wo_sb")
            nc.scalar.dma_start(
                out=wo_sb, in_=wo_ext.rearrange("(a p) n -> p a n", p=128)
            )
            for rb in range(4, 8):
                x_sb = xpool.tile([128, D], BF16, name="x_sb", tag="x_sb")
                nc.sync.dma_start(out=x_sb, in_=x_ext[ts(rb, 128), :])
                x_sbs[rb] = x_sb
            # chunks 2-3 x^T via whole-chunk DMA transposes, LAST on the
            # sync ring: Tile mutually serializes DMA transposes against all
            # other DMAs, so they must run after the startup-critical loads.
            for c in (2, 3):
                nc.sync.dma_start(
                    out=xT[:, :, ts(c, QCH)],
                    in_=x_ext[ts(c, QCH), :], transpose=True,
                )
            # warm the exp table set during the preamble
            nc.vector.memset(warm_sb, 0.0)
            nc.scalar.activation(
                warm_sb, warm_sb, mybir.ActivationFunctionType.Exp,
            )

            # ---- context transpose + K/V projections ----------------------
            for a in range(2):
                tp = mpsum.tile([128, KCH, 128], BF16, name="tp_b", tag="ps")
                for k in range(KCH):
                    nc.tensor.transpose(
                        tp[:, k, :], ctx_sb[:, a, ts(k, 128)], identb
                    )
                nc.vector.tensor_copy(ctxT[:, :, ts(a, 128)], tp)

            def kproj_group(m):
                ps = mpsum.tile([128, M], F32, name="ps_k", tag="ps")
                for k in range(KCH):
                    nc.tensor.matmul(
                        ps[:, :], wk_sb[:, k, ts(m, 128)], ctxT[:, k, :],
                        start=(k == 0), stop=(k == KCH - 1),
                    )
                nc.vector.tensor_copy(kT[:, m, :], ps)

            def vproj_group(j, n):
                psv = mpsum.tile([128, 512], F32, name="ps_v", tag="ps")
                for k in range(KCH):
                    nc.tensor.matmul(
                        psv[:, :], ctxT[:, k, ts(j, 128)],
                        wv_sb[:, k, ts(n, 512)],
                        start=(k == 0), stop=(k == KCH - 1),
                    )
                nc.vector.tensor_copy(vvs[:, j, 8 * n : 8 * n + 8, 0:HD], psv)

            def xtr_group(rb):
                # x row-block rb -> x^T columns, on the PE
                x_sb = x_sbs.pop(rb)
                tp = mpsum.tile([128, DT, 128], BF16, name="tp_x", tag="ps")
                for kk in range(DT):
                    nc.tensor.transpose(
                        tp[:, kk, :], x_sb[:, ts(kk, 128)], identb
                    )
                if rb % 2 == 0:
                    nc.vector.tensor_copy(xT[:, :, ts(rb, 128)], tp)
                else:
                    nc.scalar.copy(xT[:, :, ts(rb, 128)], tp)

            # ---- PE work groups -------------------------------------------
            def qproj_group(c, m):
                ps = mpsum.tile([128, QCH], F32, name="ps_q", tag="ps")
                for k in range(DT):
                    nc.tensor.matmul(
                        ps[:, :], wq_sb[:, k, ts(m, 128)],
                        xT[:, k, ts(c, QCH)],
                        start=(k == 0), stop=(k == DT - 1),
                    )
                nc.vector.tensor_copy(qT[:, m, ts(c, QCH)], ps)

            ob_open = {}

            def outproj_group(c, mr, n):
                mq = 4 * c + mr
                ps = mpsum.tile([128, 512], F32, name="ps_o", tag="ps")
                for k in range(DT):
                    nc.tensor.matmul(
                        ps[:, :], oT[:, k, ts(mq, 128)],
                        wo_sb[:, k, ts(n, 512)],
                        start=(k == 0), stop=(k == DT - 1),
                    )
                # both 512-halves land in one [128, 1024] tile so the row
                # block ships as a single 256KB store
                if n == 0:
                    ob = outp.tile([128, D], BF16, name="ob", tag="ob")
                    ob_open[mq] = ob
                    nc.scalar.copy(ob[:, 0:512], ps)
                else:
                    ob = ob_open.pop(mq)
                    nc.vector.tensor_copy(ob[:, 512:D], ps)
                    nc.sync.dma_start(out=out_ext[ts(mq, 128), :], in_=ob)

            pending = []

            def drain(n=None):
                k = len(pending) if n is None else min(n, len(pending))
                for _ in range(k):
                    pending.pop(0)()

            # minimal preamble: only what attention i=0 of chunk 0 needs;
            # the rest of the projections drain just-in-time inside the
            # chunk-0 spine (3 filler groups per i keep every dependency one
            # head-pair ahead of its consumer).
            kproj_group(0)
            for rb in range(4):
                xtr_group(rb)
            qproj_group(0, 0)
            vproj_group(0, 0)
            vproj_group(1, 0)

            # ---- main loop over q chunks ----------------------------------
            for c in range(NQC):
                if c == 0:
                    for m in range(1, DT):
                        pending.append(lambda m=m: qproj_group(0, m))
                        pending.append(lambda m=m: kproj_group(m))
                        if m == 3:
                            pending.append(lambda: vproj_group(0, 1))
                            pending.append(lambda: vproj_group(1, 1))
                if c == 0:
                    for rb in range(4, 8):
                        pending.append(lambda rb=rb: xtr_group(rb))
                if c + 1 < NQC:
                    for m in range(DT):
                        pending.append(
                            lambda c=c + 1, m=m: qproj_group(c, m)
                        )
                if c >= 1:
                    for mr in range(4):
                        for n in range(2):
                            pending.append(
                                lambda c=c - 1, mr=mr, n=n: outproj_group(c, mr, n)
                            )
                norm_prev = None
                for i in range(DT):
                    # scores + exp for head pair i
                    attns = []
                    for j in range(2):  # kv chunk
                        sc_h = mpsum.tile([128, QCH], F32, name="sc_h", tag="ps")
                        sc_p = mpsum.tile([128, QCH], F32, name="sc_p", tag="ps")
                        nc.tensor.matmul(
                            sc_h[:, :], kT[0:64, i, ts(j, 128)],
                            qT[0:64, i, ts(c, QCH)],
                            start=True, stop=True, tile_position=(0, 0),
                        )
                        nc.tensor.matmul(
                            sc_p[:, :], kT[64:128, i, ts(j, 128)],
                            qT[64:128, i, ts(c, QCH)],
                            start=True, stop=True, tile_position=(64, 0),
                        )
                        at_h = attnp.tile([128, QCH], BF16, name="at_h", tag="at_h")
                        at_p = attnp.tile([128, QCH], BF16, name="at_p", tag="at_p")
                        nc.scalar.activation(
                            at_h, sc_h, mybir.ActivationFunctionType.Exp,
                            bias=mask_sb[:, j : j + 1], scale=SCALE,
                        )
                        nc.scalar.activation(
                            at_p, sc_p, mybir.ActivationFunctionType.Exp,
                            bias=mask_sb[:, j : j + 1], scale=SCALE,
                        )
                        attns.append((at_h, at_p))
                    # PE filler while exp runs. Chunk 0 drains 3 groups per
                    # head-pair so its just-in-time K/Q projections stay one
                    # step ahead of the attention consuming them.
                    drain(3 if c == 0 else (2 if i == 0 else 1))
                    # AV with fused sums (65th stationary column of ones)
                    av_h = mpsum.tile([128, QCH], F32, name="av_h", tag="av_h",
                                      bufs=2)
                    av_p = mpsum.tile([128, QCH], F32, name="av_p", tag="av_p",
                                      bufs=2)
                    for j in range(2):
                        at_h, at_p = attns[j]
                        nc.tensor.matmul(
                            av_h[0 : HD + 1, :], vvs[:, j, 2 * i, :], at_h,
                            start=(j == 0), stop=(j == 1),
                        )
                        nc.tensor.matmul(
                            av_p[0 : HD + 1, :], vvs[:, j, 2 * i + 1, :], at_p,
                            start=(j == 0), stop=(j == 1),
                        )
                    # gather the two sum rows (cast to bf16 for the selector)
                    srows = srA if i % 2 == 0 else srB
                    nc.scalar.copy(srows[0:1, :], av_h[HD : HD + 1, :])
                    nc.vector.tensor_copy(
                        srows[64:65, :], av_p[HD : HD + 1, :]
                    )
                    if norm_prev is not None:
                        norm_prev()

                    def make_norm(c=c, i=i, av_h=av_h, av_p=av_p, srows=srows):
                        def norm():
                            rec_ps = mpsum.tile([128, QCH], F32, name="rec_ps",
                                                tag="rec_ps", bufs=1)
                            nc.tensor.matmul(
                                rec_ps[:, :], sel2[:, :], srows[:, :],
                                start=True, stop=True,
                            )
                            rec = recp.tile([128, QCH], F32, name="rec", tag="rec")
                            nc.vector.reciprocal_approx_fast(rec, rec_ps)
                            nc.vector.tensor_mul(
                                oT[0:64, i, ts(c, QCH)], av_h[0:64, :], rec[0:64, :]
                            )
                            nc.vector.tensor_mul(
                                oT[64:128, i, ts(c, QCH)], av_p[0:64, :],
                                rec[64:128, :]
                            )
                        return norm

                    norm_prev = make_norm()
                drain(1)
                norm_prev()
                drain()

            for mr in range(4):
                for n in range(2):
                    outproj_group(NQC - 1, mr, n)

        # release singles in reverse allocation order
        free_oT()
        free_qT()
        free_xT()
        free_vvs()
        free_kT()
        free_srB()
        free_srA()
        free_sel2()
        free_warm()
        free_ones11()
        free_maskr()
        free_mask()
        free_identb()

    nc.finalize()
    return nc


_NC_CACHE = None


def _get_nc():
    global _NC_CACHE
    if _NC_CACHE is None:
        _NC_CACHE = build_nc()
    return _NC_CACHE


def kernel(x, context, context_mask, Wq, Wk, Wv, Wo):
    import ml_dtypes

    bf = ml_dtypes.bfloat16
    x = np.ascontiguousarray(np.asarray(x).astype(bf))
    context = np.ascontiguousarray(np.asarray(context).astype(bf))
    Wq = np.ascontiguousarray(np.asarray(Wq).astype(bf))
    Wk = np.ascontiguousarray(np.asarray(Wk).astype(bf))
    Wv = np.ascontiguousarray(np.asarray(Wv).astype(bf))
    Wo = np.ascontiguousarray(np.asarray(Wo).astype(bf))
    mask = np.asarray(context_mask)

    # additive exp-bias per kv position: 0 where visible, -1e9 where masked
    bias = (mask.astype(np.float32) - 1.0) * 1e9          # [B, M]
    x_flat = x.reshape(B * N, D)

    nc = _get_nc()
    in_maps = []
    for c in range(N_CORES):
        b = (c * NQ) // N
        in_maps.append({
            "x": x_flat[c * NQ : (c + 1) * NQ],
            "ctx": context[b],
            "maskb": np.ascontiguousarray(bias[b].reshape(1, M)),
            "wq": Wq, "wk": Wk, "wv": Wv, "wo": Wo,
        })
    res = run_bass_kernel_spmd(nc, in_maps, core_ids=list(range(N_CORES)))
    out = np.concatenate(
        [np.asarray(res.results[c]["out"]) for c in range(N_CORES)], axis=0
    )
    return out.astype(np.float32).reshape(B, N, D)


# revision 72
# speedup vs baseline: 1.0116x; 1.0116x over previous
"""Cross-attention kernel for 8 TRN2 NeuronCores (v2).

Reference computation (per problem spec):
    q = (x @ Wq)  [B=4, N=4096, D=1024] -> heads [B, 16, N, 64]
    k = (context @ Wk), v = (context @ Wv)   context [B, M=256, 768]
    out = softmax(q k^T / 8 + mask) v   -> [B, N, D] @ Wo

Sharding: the 16384 query rows (B*N) are split evenly across the 8 cores
(2048 rows each, each shard living inside one batch). K/V are computed
redundantly per core from that core's batch context (only ~0.8 GFLOP) so no
collectives are needed; each core produces its own 2048 output rows and the
host concatenates them.

v2 changes vs v1 (282.9us -> ~239us):
  * softmax sums come free from the AV matmul via an all-ones 65th column
    appended to each V-head stationary (kills the 128 ones-matmuls, ~27us
    of PE streaming); the per-(head,q) sums are re-spread across
    partitions with a tiny selector matmul, then reciprocal + normalize.
  * the attention spine is exp(ScalarE)-co-limited, so Qproj of chunk c+1,
    outproj of chunk c-1, and the K/V/Q projections of chunk 0 are
    interleaved into it as just-in-time PE filler groups.
  * x^T: PE-transposed for chunks 0-1 (startup-critical), whole-chunk DMA
    transposes for chunks 2-3 placed last on the sync ring (DMA-transpose
    descriptor generation is ~8us and Tile mutually serializes it against
    every other DMA, so it must stay off the startup path).
  * input DMAs split across sync/scalar rings in consumption order, with
    wk/wq/wv halved so the first half unblocks the PE early; the mask
    rides in as one [1,256] row and is spread across partitions by two
    1-column matmuls (the [128,2] layout's 8-byte descriptors were slow).
  * output stored bf16 (halves store traffic); host upcasts.
"""

import sys

for _p in ("/opt/trn_rl_repo",):
    if _p not in sys.path:
        sys.path.insert(0, _p)

import numpy as np

import concourse.bass as bass
import concourse.mybir as mybir
import concourse.tile as tile
from concourse.masks import make_identity
from concourse import bacc
from concourse.bass_utils import run_bass_kernel_spmd

ts = bass.ts

N_CORES = 8
B, N, D = 4, 4096, 1024
CTX = 768
M = 256          # kv length
H, HD = 16, 64   # heads, head dim
NQ = (B * N) // N_CORES   # 2048 query rows per core
QCH = 512                 # q chunk (free dim of most matmuls)
NQC = NQ // QCH           # 4 q chunks
DT = D // 128             # 8 d-blocks (= head pairs)
KCH = CTX // 128          # 6 contraction chunks for context projections
F32 = mybir.dt.float32
BF16 = mybir.dt.bfloat16

SCALE = HD ** -0.5


def build_nc():
    nc = bacc.Bacc()

    x_ext = nc.declare_dram_parameter("x", [NQ, D], BF16, isOutput=False)
    ctx_ext = nc.declare_dram_parameter("ctx", [M, CTX], BF16, isOutput=False)
    maskb_ext = nc.declare_dram_parameter("maskb", [1, M], F32, isOutput=False)
    wq_ext = nc.declare_dram_parameter("wq", [D, D], BF16, isOutput=False)
    wk_ext = nc.declare_dram_parameter("wk", [CTX, D], BF16, isOutput=False)
    wv_ext = nc.declare_dram_parameter("wv", [CTX, D], BF16, isOutput=False)
    wo_ext = nc.declare_dram_parameter("wo", [D, D], BF16, isOutput=False)
    out_ext = nc.declare_dram_parameter("out", [NQ, D], BF16, isOutput=True)

    with tile.TileContext(nc) as tc:
        # ---- persistent tensors -------------------------------------------
        identb, free_identb = tc.tile([128, 128], BF16, name="identb")
        make_identity(nc, identb)
        mask_sb, free_mask = tc.tile([128, 2], F32, name="mask_sb")
        maskr_sb, free_maskr = tc.tile([1, M], F32, name="maskr_sb")
        ones11, free_ones11 = tc.tile([1, 1], F32, name="ones11")
        warm_sb, free_warm = tc.tile([128, 1], F32, name="warm_sb")
        # selector: row 0 -> output partitions 0-63 (head 2i sums), row 64 ->
        # partitions 64-127 (head 2i+1 sums); all other rows zero.
        sel2, free_sel2 = tc.tile([128, 128], BF16, name="sel2")
        nc.vector.memset(sel2, 0.0)
        nc.vector.memset(sel2[0:1, 0:64], 1.0)
        nc.vector.memset(sel2[64:65, 64:128], 1.0)
        # double-buffered sum-row carriers (only rows 0 and 64 ever written)
        srA, free_srA = tc.tile([128, QCH], BF16, name="srA")
        srB, free_srB = tc.tile([128, QCH], BF16, name="srB")
        nc.vector.memset(srA, 0.0)
        nc.vector.memset(srB, 0.0)

        kT, free_kT = tc.tile([128, DT, M], BF16, name="kT")
        # V stationaries: per (kv-half j, head h) a [128, 65] block whose
        # 65th column is all-ones so the AV matmul also emits softmax sums.
        vvs, free_vvs = tc.tile([128, 2, H, HD + 1], BF16, name="vvs")
        nc.gpsimd.memset(vvs[:, :, :, HD : HD + 1], 1.0)
        xT, free_xT = tc.tile([128, DT, NQ], BF16, name="xT")
        qT, free_qT = tc.tile([128, DT, NQ], BF16, name="qT")
        oT, free_oT = tc.tile([128, DT, NQ], BF16, name="oT")

        with tc.tile_pool(name="weights", bufs=1) as wpool, \
             tc.tile_pool(name="bpool", bufs=1) as bpool, \
             tc.tile_pool(name="xpool", bufs=8) as xpool, \
             tc.tile_pool(name="attnp", bufs=3) as attnp, \
             tc.tile_pool(name="recp", bufs=2) as recp, \
             tc.tile_pool(name="outp", bufs=4) as outp, \
             tc.tile_pool(name="mpsum", bufs=3, space="PSUM") as mpsum:
            # ---- input DMAs (no DMA transposes: Tile mutually serializes
            # them against every other DMA, which stalls the weight loads).
            # sync ring: mask, ctx, wv, x row-blocks; scalar: wq, wk, wo.
            # mask arrives as one contiguous [1, 256] row (a single fast
            # descriptor); a pair of 1-column matmuls spreads it across
            # partitions into the [128, 2] per-partition bias layout.
            nc.sync.dma_start(out=maskr_sb, in_=maskb_ext[:, :])
            nc.vector.memset(ones11, 1.0)
            # p-state warm-up: the PE starts at the 1.2GHz pstate and only
            # ramps with activity (early matmuls measured ~2x slower). Burn
            # the DMA-wait window with data-free transposes so the clock is
            # at full rate when real work arrives.
            for w in range(4):
                tpw = mpsum.tile([128, DT, 128], BF16, name="tp_w", tag="ps")
                for kk in range(DT):
                    nc.tensor.transpose(tpw[:, kk, :], identb, identb)
            mps = mpsum.tile([128, 2], F32, name="mps", tag="ps")
            for j in range(2):
                nc.tensor.matmul(
                    mps[:, j : j + 1], maskr_sb[0:1, ts(j, 128)], ones11,
                    start=True, stop=True,
                )
            nc.vector.tensor_copy(mask_sb, mps)
            # ctx plain load in two halves (DMA-transpose desc-gen is ~7us
            # and blocks the ring; the PE transposes it in ~0.7us instead)
            ctx_sb = bpool.tile([128, 2, CTX], BF16, name="ctx_sb")
            ctx_r = ctx_ext.rearrange("(a p) n -> p a n", p=128)
            nc.sync.dma_start(out=ctx_sb[:, 0:1, :], in_=ctx_r[:, 0:1, :])
            nc.sync.dma_start(out=ctx_sb[:, 1:2, :], in_=ctx_r[:, 1:2, :])
            ctxT = bpool.tile([128, KCH, M], BF16, name="ctxT")
            # weights split into column halves so the first half unblocks
            # the PE early; wk/wq on scalar, wv on sync.
            wk_sb = bpool.tile([128, KCH, D], BF16, name="wk_sb")
            wq_sb = wpool.tile([128, DT, D], BF16, name="wq_sb")
            wv_sb = bpool.tile([128, KCH, D], BF16, name="wv_sb")
            wk_r = wk_ext.rearrange("(a p) n -> p a n", p=128)
            wq_r = wq_ext.rearrange("(a p) n -> p a n", p=128)
            wv_r = wv_ext.rearrange("(a p) n -> p a n", p=128)
            nc.scalar.dma_start(out=wk_sb[:, :, 0:512], in_=wk_r[:, :, 0:512])
            nc.scalar.dma_start(out=wq_sb[:, :, 0:512], in_=wq_r[:, :, 0:512])
            # x row-blocks for chunks 0-1 (plain loads; PE transposes them)
            x_sbs = {}
            for rb in range(4):
                x_sb = xpool.tile([128, D], BF16, name="x_sb", tag="x_sb")
                nc.sync.dma_start(out=x_sb, in_=x_ext[ts(rb, 128), :])
                x_sbs[rb] = x_sb
            nc.sync.dma_start(out=wv_sb[:, :, 0:512], in_=wv_r[:, :, 0:512])
            nc.scalar.dma_start(out=wk_sb[:, :, 512:D], in_=wk_r[:, :, 512:D])
            nc.scalar.dma_start(out=wq_sb[:, :, 512:D], in_=wq_r[:, :, 512:D])
            nc.sync.dma_start(out=wv_sb[:, :, 512:D], in_=wv_r[:, :, 512:D])
            wo_sb = wpool.tile([128, DT, D], BF16, name="wo_sb")
            nc.scalar.dma_start(
                out=wo_sb, in_=wo_ext.rearrange("(a p) n -> p a n", p=128)
            )
            for rb in range(4, 8):
                x_sb = xpool.tile([128, D], BF16, name="x_sb", tag="x_sb")
                nc.sync.dma_start(out=x_sb, in_=x_ext[ts(rb, 128), :])
                x_sbs[rb] = x_sb
            # chunks 2-3 x^T via whole-chunk DMA transposes, LAST on the
            # sync ring: Tile mutually serializes DMA transposes against all
            # other DMAs, so they must run after the startup-critical loads.
            for c in (2, 3):
                nc.sync.dma_start(
                    out=xT[:, :, ts(c, QCH)],
                    in_=x_ext[ts(c, QCH), :], transpose=True,
                )
            # warm the exp table set during the preamble
            nc.vector.memset(warm_sb, 0.0)
            nc.scalar.activation(
                warm_sb, warm_sb, mybir.ActivationFunctionType.Exp,
            )

            # ---- context transpose + K/V projections ----------------------
            for a in range(2):
                tp = mpsum.tile([128, KCH, 128], BF16, name="tp_b", tag="ps")
                for k in range(KCH):
                    nc.tensor.transpose(
                        tp[:, k, :], ctx_sb[:, a, ts(k, 128)], identb
                    )
                nc.vector.tensor_copy(ctxT[:, :, ts(a, 128)], tp)

            def kproj_group(m):
                ps = mpsum.tile([128, M], F32, name="ps_k", tag="ps")
                for k in range(KCH):
                    nc.tensor.matmul(
                        ps[:, :], wk_sb[:, k, ts(m, 128)], ctxT[:, k, :],
                        start=(k == 0), stop=(k == KCH - 1),
                    )
                nc.vector.tensor_copy(kT[:, m, :], ps)

            def vproj_group(j, n):
                psv = mpsum.tile([128, 512], F32, name="ps_v", tag="ps")
                for k in range(KCH):
                    nc.tensor.matmul(
                        psv[:, :], ctxT[:, k, ts(j, 128)],
                        wv_sb[:, k, ts(n, 512)],
                        start=(k == 0), stop=(k == KCH - 1),
                    )
                nc.vector.tensor_copy(vvs[:, j, 8 * n : 8 * n + 8, 0:HD], psv)

            def xtr_group(rb):
                # x row-block rb -> x^T columns, on the PE
                x_sb = x_sbs.pop(rb)
                tp = mpsum.tile([128, DT, 128], BF16, name="tp_x", tag="ps")
                for kk in range(DT):
                    nc.tensor.transpose(
                        tp[:, kk, :], x_sb[:, ts(kk, 128)], identb
                    )
                if rb % 2 == 0:
                    nc.vector.tensor_copy(xT[:, :, ts(rb, 128)], tp)
                else:
                    nc.scalar.copy(xT[:, :, ts(rb, 128)], tp)

            # ---- PE work groups -------------------------------------------
            def qproj_group(c, m):
                ps = mpsum.tile([128, QCH], F32, name="ps_q", tag="ps")
                for k in range(DT):
                    nc.tensor.matmul(
                        ps[:, :], wq_sb[:, k, ts(m, 128)],
                        xT[:, k, ts(c, QCH)],
                        start=(k == 0), stop=(k == DT - 1),
                    )
                nc.vector.tensor_copy(qT[:, m, ts(c, QCH)], ps)

            def outproj_group(c, mr, n):
                mq = 4 * c + mr
                ps = mpsum.tile([128, 512], F32, name="ps_o", tag="ps")
                for k in range(DT):
                    nc.tensor.matmul(
                        ps[:, :], oT[:, k, ts(mq, 128)],
                        wo_sb[:, k, ts(n, 512)],
                        start=(k == 0), stop=(k == DT - 1),
                    )
                ob = outp.tile([128, 512], BF16, name="ob", tag="ob")
                if n == 0:
                    nc.scalar.copy(ob, ps)
                else:
                    nc.vector.tensor_copy(ob, ps)
                nc.sync.dma_start(
                    out=out_ext[ts(mq, 128), ts(n, 512)], in_=ob
                )

            pending = []

            def drain(n=None):
                k = len(pending) if n is None else min(n, len(pending))
                for _ in range(k):
                    pending.pop(0)()

            # minimal preamble: only what attention i=0 of chunk 0 needs;
            # the rest of the projections drain just-in-time inside the
            # chunk-0 spine (3 filler groups per i keep every dependency one
            # head-pair ahead of its consumer).
            kproj_group(0)
            for rb in range(4):
                xtr_group(rb)
            qproj_group(0, 0)
            vproj_group(0, 0)
            vproj_group(1, 0)

            # ---- main loop over q chunks ----------------------------------
            for c in range(NQC):
                if c == 0:
                    for m in range(1, DT):
                        pending.append(lambda m=m: qproj_group(0, m))
                        pending.append(lambda m=m: kproj_group(m))
                        if m == 3:
                            pending.append(lambda: vproj_group(0, 1))
                            pending.append(lambda: vproj_group(1, 1))
                if c == 0:
                    for rb in range(4, 8):
                        pending.append(lambda rb=rb: xtr_group(rb))
                if c + 1 < NQC:
                    for m in range(DT):
                        pending.append(
                            lambda c=c + 1, m=m: qproj_group(c, m)
                        )
                if c >= 1:
                    for mr in range(4):
                        for n in range(2):
                            pending.append(
                                lambda c=c - 1, mr=mr, n=n: outproj_group(c, mr, n)
                            )
                norm_prev = None
                for i in range(DT):
                    # scores + exp for head pair i
                    attns = []
                    for j in range(2):  # kv chunk
                        sc_h = mpsum.tile([128, QCH], F32, name="sc_h", tag="ps")
                        sc_p = mpsum.tile([128, QCH], F32, name="sc_p", tag="ps")
                        nc.tensor.matmul(
                            sc_h[:, :], kT[0:64, i, ts(j, 128)],
                            qT[0:64, i, ts(c, QCH)],
                            start=True, stop=True, tile_position=(0, 0),
                        )
                        nc.tensor.matmul(
                            sc_p[:, :], kT[64:128, i, ts(j, 128)],
                            qT[64:128, i, ts(c, QCH)],
                            start=True, stop=True, tile_position=(64, 0),
                        )
                        at_h = attnp.tile([128, QCH], BF16, name="at_h", tag="at_h")
                        at_p = attnp.tile([128, QCH], BF16, name="at_p", tag="at_p")
                        nc.scalar.activation(
                            at_h, sc_h, mybir.ActivationFunctionType.Exp,
                            bias=mask_sb[:, j : j + 1], scale=SCALE,
                        )
                        nc.scalar.activation(
                            at_p, sc_p, mybir.ActivationFunctionType.Exp,
                            bias=mask_sb[:, j : j + 1], scale=SCALE,
                        )
                        attns.append((at_h, at_p))
                    # PE filler while exp runs. Chunk 0 drains 3 groups per
                    # head-pair so its just-in-time K/Q projections stay one
                    # step ahead of the attention consuming them.
                    drain(3 if c == 0 else (2 if i == 0 else 1))
                    # AV with fused sums (65th stationary column of ones)
                    av_h = mpsum.tile([128, QCH], F32, name="av_h", tag="av_h",
                                      bufs=2)
                    av_p = mpsum.tile([128, QCH], F32, name="av_p", tag="av_p",
                                      bufs=2)
                    for j in range(2):
                        at_h, at_p = attns[j]
                        nc.tensor.matmul(
                            av_h[0 : HD + 1, :], vvs[:, j, 2 * i, :], at_h,
                            start=(j == 0), stop=(j == 1),
                        )
                        nc.tensor.matmul(
                            av_p[0 : HD + 1, :], vvs[:, j, 2 * i + 1, :], at_p,
                            start=(j == 0), stop=(j == 1),
                        )
                    # gather the two sum rows (cast to bf16 for the selector)
                    srows = srA if i % 2 == 0 else srB
                    nc.scalar.copy(srows[0:1, :], av_h[HD : HD + 1, :])
                    nc.vector.tensor_copy(
                        srows[64:65, :], av_p[HD : HD + 1, :]
                    )
                    if norm_prev is not None:
                        norm_prev()

                    def make_norm(c=c, i=i, av_h=av_h, av_p=av_p, srows=srows):
                        def norm():
                            rec_ps = mpsum.tile([128, QCH], F32, name="rec_ps",
                                                tag="rec_ps", bufs=1)
                            nc.tensor.matmul(
                                rec_ps[:, :], sel2[:, :], srows[:, :],
                                start=True, stop=True,
                            )
                            rec = recp.tile([128, QCH], F32, name="rec", tag="rec")
                            nc.vector.reciprocal_approx_fast(rec, rec_ps)
                            nc.vector.tensor_mul(
                                oT[0:64, i, ts(c, QCH)], av_h[0:64, :], rec[0:64, :]
                            )
                            nc.vector.tensor_mul(
                                oT[64:128, i, ts(c, QCH)], av_p[0:64, :],
                                rec[64:128, :]
                            )
                        return norm

                    norm_prev = make_norm()
                drain(1)
                norm_prev()
                drain()

            for mr in range(4):
                for n in range(2):
                    outproj_group(NQC - 1, mr, n)

        # release singles in reverse allocation order
        free_oT()
        free_qT()
        free_xT()
        free_vvs()
        free_kT()
        free_srB()
        free_srA()
        free_sel2()
        free_warm()
        free_ones11()
        free_maskr()
        free_mask()
        free_identb()

    nc.finalize()
    return nc


_NC_CACHE = None


def _get_nc():
    global _NC_CACHE
    if _NC_CACHE is None:
        _NC_CACHE = build_nc()
    return _NC_CACHE


def kernel(x, context, context_mask, Wq, Wk, Wv, Wo):
    import ml_dtypes

    bf = ml_dtypes.bfloat16
    x = np.ascontiguousarray(np.asarray(x).astype(bf))
    context = np.ascontiguousarray(np.asarray(context).astype(bf))
    Wq = np.ascontiguousarray(np.asarray(Wq).astype(bf))
    Wk = np.ascontiguousarray(np.asarray(Wk).astype(bf))
    Wv = np.ascontiguousarray(np.asarray(Wv).astype(bf))
    Wo = np.ascontiguousarray(np.asarray(Wo).astype(bf))
    mask = np.asarray(context_mask)

    # additive exp-bias per kv position: 0 where visible, -1e9 where masked
    bias = (mask.astype(np.float32) - 1.0) * 1e9          # [B, M]
    x_flat = x.reshape(B * N, D)

    nc = _get_nc()
    in_maps = []
    for c in range(N_CORES):
        b = (c * NQ) // N
        in_maps.append({
            "x": x_flat[c * NQ : (c + 1) * NQ],
            "ctx": context[b],
            "maskb": np.ascontiguousarray(bias[b].reshape(1, M)),
            "wq": Wq, "wk": Wk, "wv": Wv, "wo": Wo,
        })
    res = run_bass_kernel_spmd(nc, in_maps, core_ids=list(range(N_CORES)))
    out = np.concatenate(
        [np.asarray(res.results[c]["out"]) for c in range(N_CORES)], axis=0
    )
    return out.astype(np.float32).reshape(B, N, D)
